# revision 17
# baseline (speedup 1.0000x reference)
"""Trainium2 Bass kernel for nn_MedianPool2d (K=3, stride=1, same-pad along W).

The reference op is a width-wise median-of-3 with replicate padding:
    out[..., w] = median(x[..., w-1], x[..., w], x[..., w+1])   (clamped at edges)

Strategy (fp16 internal; gate is rel_err < 2e-2, we land at ~4.7e-4):
  - Shard batch across 8 NeuronCores (4 batches/core), fully data parallel.
    Host converts fp32 <-> fp16 so the device moves half the HBM bytes.
  - The DVE's 4-op min/max network (2 fp16/cycle/lane in 2x_1P mode) is a hard
    ~136us/core floor for all 8.4M elements: tensor_tensor has no 4x uop, no
    dual-output uop, custom DVE Specs run at 1x, GPSIMD is locked out of its
    shared SBUF port during every 2-port DVE op, and ScalarE has no two-tensor
    ops. So the kernel splits rows between two independent median pipelines:
      * DVE path (51/64 of rows): lo=min(x,S), hi=max(x,S), tm=min(hi,x+2),
        out=max(lo,tm), with S=x+1 an ACT shift copy that keeps every TT
        operand 4B-aligned (2x mode needs 16-bit + step 1 + 4B-aligned APs).
        Ops use per-row 3D APs [P, r, W-2] (row stride 2048B keeps alignment),
        so no cross-row seam garbage and no seam fixups.
      * PE/ACT path (13/64 of rows), on the otherwise-idle TensorE + spare
        ScalarE:  r0 = relu(c - a);  out = b + relu(c-b-r0) - relu(b-a-r0)
        (= clamp(b, min(a,c), max(a,c)) = median3). Linear terms are identity
        matmuls (+I/-I [128x128] fp16 consts) accumulated in PSUM, 11 matmuls
        per row in 511-wide chunks; relus + final fp32->fp16 cast-copy are 4
        ACT instrs per row reading PSUM. 22% of rows costs PE ~5.9us/row-unit
        and ACT ~4us, balancing DVE ~2.1us and ACT-copy ~0.9us per unit.
  - Engine budgets/core (sustained): DVE ~109, ACT ~100, PE ~64, DMA ~108
    (load+store 32MB at ~295GB/s sustained HBM; ~350GB/s burst).
  - The +1 output shift is absorbed by the store DMAs (byte-granular): ot rows
    are W+2 long so the DVE/ACT write dst starts 4B-aligned and the store
    reads at a one-element offset.  ot_dve and ot_pe are separate tiles with
    a single writer engine each (HWDGE store DMAs allow only one sem wait).
  - tc.high_priority() on the ACT shift copies and load DMAs: without it the
    shift copy queues behind the aux path's relu bursts on ACT and stalls the
    DVE (~25us). Deep xt buffering (bufs=5) rides out DMA/thermal jitter.
  - Timing is measured with the repeats-slope method; per-rep time grows with
    program length (throttling: ~88us/rep over reps 1-33, ~115 over 1-257,
    ~125 sustained at 129-257), so tile-schedule ramps amortize away and only
    engine-busy work matters.
"""

import numpy as np

import concourse.bacc as bacc
import concourse.bass as bass
import concourse.mybir as mybir
import concourse.tile as tile
from concourse.alu_op_type import AluOpType
from concourse.bass_utils import run_bass_kernel_spmd

N_CORES = 8
B, C, H, W = 32, 1, 2048, 1024
P = 128
ROWS = (B // N_CORES) * C * H  # 8192 rows per core
FP16 = mybir.dt.float16


DEFAULT_RS = (2, 2, 4, 8, 8, 8, 8, 8, 8, 4, 2, 2)  # sums to 64 row-units


def build_program(
    repeats: int = 1,
    rs: tuple = DEFAULT_RS,       # tile schedule in row-units (each unit = 128x1024)
    do_compute: bool = True,
    do_dma=True,                  # True / False / "load" (skip stores)
    copy_engine: str = "scalar",   # engine for the S shift copy
    dual_ring: bool = False,       # alternate store DMAs onto the ACT ring
    dual_load: bool = False,       # alternate load DMAs onto the ACT ring
    fixups: bool = True,           # seam/edge fixups (disable for timing only)
    bufs: tuple = (4, 2, 1, 1, 1, 3),
) -> bass.Bass:
    assert sum(rs) == ROWS // P
    nc = bacc.Bacc("TRN2", target_bir_lowering=False, debug=False)
    x_d = nc.dram_tensor("x", [ROWS, W], FP16, kind="ExternalInput").ap()
    y_d = nc.dram_tensor("y", [ROWS, W], FP16, kind="ExternalOutput").ap()

    with tile.TileContext(nc) as tc:
        with (
            tc.tile_pool(name="xt", bufs=bufs[0]) as xpool,
            tc.tile_pool(name="st", bufs=bufs[1]) as spool,
            tc.tile_pool(name="lo", bufs=bufs[2]) as lpool,
            tc.tile_pool(name="hi", bufs=bufs[3]) as hpool,
            tc.tile_pool(name="tm", bufs=bufs[4]) as tpool,
            tc.tile_pool(name="ot", bufs=bufs[5]) as opool,
        ):
            for _rep in range(repeats):
                row0 = 0
                for t, r in enumerate(rs):
                    L = r * W
                    n = L - 2  # even op length
                    rows = slice(row0 * P, (row0 + r) * P)
                    row0 += r
                    src = x_d[rows, :].rearrange("(p r) w -> p (r w)", p=P)
                    dst = y_d[rows, :].rearrange("(p r) w -> p (r w)", p=P)

                    xt = xpool.tile([P, L], FP16, tag="xt")
                    if do_dma:
                        leng = nc.scalar if (dual_load and t % 2) else nc.sync
                        leng.dma_start(out=xt[:], in_=src)

                    if not do_compute:
                        if do_dma is True:
                            eng = nc.scalar if (dual_ring and t % 2) else nc.sync
                            eng.dma_start(out=dst, in_=xt[:])
                        continue

                    st = spool.tile([P, L], FP16, tag="st")
                    # S[i] = x[i+1]; ops only read S[0 : L-2]
                    ceng = {"scalar": nc.scalar, "gpsimd": nc.gpsimd,
                            "vector": nc.vector}[copy_engine]
                    if copy_engine == "scalar":
                        ceng.copy(out=st[:, 0 : L - 1], in_=xt[:, 1:L])
                    else:
                        ceng.tensor_copy(out=st[:, 0 : L - 1], in_=xt[:, 1:L])

                    lo = lpool.tile([P, n], FP16, tag="lo")
                    hi = hpool.tile([P, n], FP16, tag="hi")
                    tm = tpool.tile([P, n], FP16, tag="tm")
                    ot = opool.tile([P, L + 2], FP16, tag="ot")

                    eng = nc.vector
                    eng.tensor_tensor(out=lo[:], in0=xt[:, 0:n], in1=st[:, 0:n], op=AluOpType.min)
                    eng.tensor_tensor(out=hi[:], in0=xt[:, 0:n], in1=st[:, 0:n], op=AluOpType.max)
                    eng.tensor_tensor(out=tm[:], in0=hi[:], in1=xt[:, 2 : 2 + n], op=AluOpType.min)
                    # ot[2+i] = c[i] = O[i+1], i = 0..n-1  (aligned dst)
                    eng.tensor_tensor(out=ot[:, 2 : 2 + n], in0=lo[:], in1=tm[:], op=AluOpType.max)

                    # row-seam fixups: for k=1..r-1, O[kW-1]=x[kW-1], O[kW]=x[kW]
                    #   -> ot[kW : kW+2] = xt[kW-1 : kW+1]
                    if fixups and r > 1:
                        seam_dst = ot[:, W : (r - 1) * W + W : 1].rearrange(
                            "p (k w) -> p k w", w=W
                        )[:, :, 0:2]
                        seam_src = xt[:, W - 1 : (r - 1) * W + W - 1 : 1].rearrange(
                            "p (k w) -> p k w", w=W
                        )[:, :, 0:2]
                        eng.tensor_copy(out=seam_dst, in_=seam_src)
                    # edges: O[0]=x[0] -> ot[1]; O[L-1]=x[L-1] -> ot[L]
                    if fixups:
                        eng.tensor_copy(
                            out=ot[:, 1 : L + 1 : L - 1], in_=xt[:, 0 : L : L - 1]
                        )

                    if do_dma is True:
                        deng = nc.scalar if (dual_ring and t % 2) else nc.sync
                        deng.dma_start(out=dst, in_=ot[:, 1 : L + 1])
    nc.compile()
    return nc


def build_program2(
    repeats: int = 1,
    rs: tuple = (8,) * 8,
    do_dma=True,                  # True / False / "load"
    fixup_engine: str = "vector",  # engine for the per-row edge fixup
    store_ring: str = "sync",
    load_ring: str = "sync",
    bufs: tuple = (4, 2, 1, 1, 1, 3),
) -> bass.Bass:
    """Per-row 3D-AP variant: no seam fixups, one merged edge fixup per tile.

    Views are [P, r, *] with row stride W (2048B, 4B-aligned), so every TT
    operand keeps 2x packed mode. Output tile rows are padded to W+2 so the
    DVE write dst ([:, :, 2:W]) starts 4B-aligned; the store DMA reads at a
    one-element offset to undo the shift.
    """
    Wo = W + 2  # padded ot row length (stride 2052B, 4B-multiple)
    assert sum(rs) == ROWS // P
    nc = bacc.Bacc("TRN2", target_bir_lowering=False, debug=False)
    x_d = nc.dram_tensor("x", [ROWS, W], FP16, kind="ExternalInput").ap()
    y_d = nc.dram_tensor("y", [ROWS, W], FP16, kind="ExternalOutput").ap()
    rings = {"sync": nc.sync, "scalar": nc.scalar}

    with tile.TileContext(nc) as tc:
        with (
            tc.tile_pool(name="xt", bufs=bufs[0]) as xpool,
            tc.tile_pool(name="st", bufs=bufs[1]) as spool,
            tc.tile_pool(name="lo", bufs=bufs[2]) as lpool,
            tc.tile_pool(name="hi", bufs=bufs[3]) as hpool,
            tc.tile_pool(name="tm", bufs=bufs[4]) as tpool,
            tc.tile_pool(name="ot", bufs=bufs[5]) as opool,
        ):
            for _rep in range(repeats):
                row0 = 0
                for t, r in enumerate(rs):
                    L = r * W
                    m = W - 2  # interior outputs per row
                    rows = slice(row0 * P, (row0 + r) * P)
                    row0 += r
                    src = x_d[rows, :].rearrange("(p r) w -> p (r w)", p=P)
                    dst = y_d[rows, :].rearrange("(p r) w -> p r w", p=P)

                    xt = xpool.tile([P, L], FP16, tag="xt")
                    if do_dma:
                        rings[load_ring].dma_start(out=xt[:], in_=src)
                    x3 = xt[:].rearrange("p (r w) -> p r w", w=W)

                    st = spool.tile([P, L], FP16, tag="st")
                    # flat shift: S[i] = x[i+1] (cross-row values unused)
                    nc.scalar.copy(out=st[:, 0 : L - 1], in_=xt[:, 1:L])
                    s3 = st[:].rearrange("p (r w) -> p r w", w=W)

                    lo = lpool.tile([P, r, m], FP16, tag="lo")
                    hi = hpool.tile([P, r, m], FP16, tag="hi")
                    tm = tpool.tile([P, r, m], FP16, tag="tm")
                    ot = opool.tile([P, r, Wo], FP16, tag="ot")

                    eng = nc.vector
                    # centers w=1..W-2: a=x[w-1], b=x[w], c=x[w+1]
                    eng.tensor_tensor(out=lo[:], in0=x3[:, :, 0:m], in1=s3[:, :, 0:m], op=AluOpType.min)
                    eng.tensor_tensor(out=hi[:], in0=x3[:, :, 0:m], in1=s3[:, :, 0:m], op=AluOpType.max)
                    eng.tensor_tensor(out=tm[:], in0=hi[:], in1=x3[:, :, 2:W], op=AluOpType.min)
                    # ot[:, :, 2+i] = O[i+1], i=0..m-1 (aligned dst)
                    eng.tensor_tensor(out=ot[:, :, 2 : 2 + m], in0=lo[:], in1=tm[:], op=AluOpType.max)

                    # edges per row: O[0]=x[0] -> ot[:, :, 1]; O[W-1]=x[W-1] -> ot[:, :, W]
                    feng = {"vector": nc.vector, "gpsimd": nc.gpsimd}.get(fixup_engine)
                    edst = ot[:, :, 1 : Wo : W - 1]      # [P, r, 2]
                    esrc = x3[:, :, 0 : W : W - 1]       # [P, r, 2]
                    if feng is not None:
                        feng.tensor_copy(out=edst, in_=esrc)
                    else:
                        nc.scalar.copy(out=edst, in_=esrc)

                    if do_dma is True:
                        rings[store_ring].dma_start(out=dst, in_=ot[:, :, 1 : W + 1])
    nc.compile()
    return nc


_NC_CACHE: dict = {}


def prep_shards(x: np.ndarray) -> np.ndarray:
    """[B,C,H,W] fp32 -> [N_CORES, ROWS, W] fp16 (contiguous)."""
    x = np.asarray(x)
    assert x.shape == (B, C, H, W), x.shape
    return np.ascontiguousarray(x.reshape(N_CORES, ROWS, W)).astype(np.float16)


def run_sharded(x: np.ndarray, repeats: int = 1, builder=None, **knobs) -> np.ndarray:
    build = builder or build_program
    x16 = prep_shards(x)
    key = (build.__name__, repeats, tuple(sorted(knobs.items())))
    nc = _NC_CACHE.get(key)
    if nc is None:
        nc = _NC_CACHE[key] = build(repeats=repeats, **knobs)
    in_maps = [{"x": x16[i]} for i in range(N_CORES)]
    res = run_bass_kernel_spmd(nc, in_maps, core_ids=list(range(N_CORES))).results
    out = np.stack([res[i]["y"] for i in range(N_CORES)], axis=0)
    return out.reshape(B, C, H, W).astype(np.float32)


def build_program3(
    repeats: int = 1,
    rs: tuple = (8,) * 8,
    rps: tuple = (1, 2, 2, 1, 2, 2, 1, 2),  # PE-path rows per tile
    do_dma=True,
    bufs: tuple = (6, 2, 1, 1, 1, 2, 3),
    psum_bufs: int = 4,
    aux_bufs: int = 2,
    hp_loads: bool = True,  # prioritize load DMAs for the scheduler
    inplace_tm: bool = True,  # write tm over hi (frees the tm pool)
) -> bass.Bass:
    """Hybrid: DVE min/max network on rd rows + PE/ACT relu-formula on rp rows.

    PE path per row (interior centers w=1..W-2, split in two 511-chunks):
        r0 = relu(c - a); out = b + relu(c - b - r0) - relu(b - a - r0)
    which equals clamp(b, min(a,c), max(a,c)) = median3. All linear terms are
    identity matmuls accumulated in PSUM; relus/final-cast run on ScalarE.
    ot_pe has a single writer engine (ACT); ot_d's is the DVE.
    """
    import concourse.masks as masks

    Wo = W + 2
    m = W - 2
    h = m // 2  # 511
    assert sum(rs) == ROWS // P and len(rps) == len(rs)
    nc = bacc.Bacc("TRN2", target_bir_lowering=False, debug=False)
    x_d = nc.dram_tensor("x", [ROWS, W], FP16, kind="ExternalInput").ap()
    y_d = nc.dram_tensor("y", [ROWS, W], FP16, kind="ExternalOutput").ap()
    FP32 = mybir.dt.float32
    Relu = mybir.ActivationFunctionType.Relu
    Copy = mybir.ActivationFunctionType.Copy

    with tile.TileContext(nc) as tc:
        with (
            tc.tile_pool(name="const", bufs=1) as cpool,
            tc.tile_pool(name="xt", bufs=bufs[0]) as xpool,
            tc.tile_pool(name="st", bufs=bufs[1]) as spool,
            tc.tile_pool(name="lo", bufs=bufs[2]) as lpool,
            tc.tile_pool(name="hi", bufs=bufs[3]) as hpool,
            tc.tile_pool(name="tm", bufs=bufs[4]) as tpool,
            tc.tile_pool(name="ot", bufs=bufs[5]) as opool,
            tc.tile_pool(name="op", bufs=bufs[6]) as oppool,
            tc.tile_pool(name="ps", bufs=psum_bufs, space="PSUM") as pspool,
            tc.tile_pool(name="rl", bufs=aux_bufs) as rpool,
        ):
            ident = cpool.tile([P, P], FP16, tag="ident")
            nident = cpool.tile([P, P], FP16, tag="nident")
            masks.make_identity(nc, ident[:])
            nc.vector.tensor_scalar_mul(nident[:], ident[:], -1.0)

            for _rep in range(repeats):
                row0 = 0
                for t, (r, rp) in enumerate(zip(rs, rps)):
                    rd = r - rp
                    L = r * W
                    rows = slice(row0 * P, (row0 + r) * P)
                    row0 += r
                    src = x_d[rows, :].rearrange("(p r) w -> p (r w)", p=P)
                    dst = y_d[rows, :].rearrange("(p r) w -> p r w", p=P)

                    xt = xpool.tile([P, L], FP16, tag="xt")
                    if do_dma:
                        if hp_loads:
                            with tc.high_priority():
                                nc.sync.dma_start(out=xt[:], in_=src)
                        else:
                            nc.sync.dma_start(out=xt[:], in_=src)
                    x3 = xt[:].rearrange("p (r w) -> p r w", w=W)

                    # ---- DVE path: rows [0, rd) ----
                    if rd > 0:
                        Ld = rd * W
                        st = spool.tile([P, Ld], FP16, tag="st")
                        # priority 0: never let the DVE-feeding shift copy queue
                        # behind the aux path's relu bursts on the ACT engine
                        with tc.high_priority():
                            nc.scalar.copy(out=st[:, 0 : Ld], in_=xt[:, 1 : Ld + 1])
                        s3 = st[:].rearrange("p (r w) -> p r w", w=W)

                        lo = lpool.tile([P, rd, m], FP16, tag="lo")
                        hi = hpool.tile([P, rd, m], FP16, tag="hi")
                        tm = hi if inplace_tm else tpool.tile([P, rd, m], FP16, tag="tm")
                        ot = opool.tile([P, rd, Wo], FP16, tag="ot")

                        eng = nc.vector
                        xd = x3[:, 0:rd]
                        sd = s3[:, 0:rd]
                        eng.tensor_tensor(out=lo[:], in0=xd[:, :, 0:m], in1=sd[:, :, 0:m], op=AluOpType.min)
                        eng.tensor_tensor(out=hi[:], in0=xd[:, :, 0:m], in1=sd[:, :, 0:m], op=AluOpType.max)
                        eng.tensor_tensor(out=tm[:], in0=hi[:], in1=xd[:, :, 2:W], op=AluOpType.min)
                        eng.tensor_tensor(out=ot[:, :, 2 : 2 + m], in0=lo[:], in1=tm[:], op=AluOpType.max)
                        eng.tensor_copy(out=ot[:, :, 1 : Wo : W - 1], in_=xd[:, :, 0 : W : W - 1])
                        if do_dma is True:
                            nc.sync.dma_start(out=dst[:, 0:rd], in_=ot[:, :, 1 : W + 1])

                    # ---- PE path: rows [rd, r) ----
                    if rp > 0:
                        otp = oppool.tile([P, rp, Wo], FP16, tag="otp")
                        for j in range(rd, r):
                            xr = x3[:, j]          # [P, W]
                            # sub-chunk views: centers w in [1+h*sc, 1+h*(sc+1))
                            def av(sc):
                                return xr[:, h * sc : h * sc + h]
                            def bv(sc):
                                return xr[:, 1 + h * sc : 1 + h * sc + h]
                            def cv(sc):
                                return xr[:, 2 + h * sc : 2 + h * sc + h]

                            psD = pspool.tile([P, 2, 512], FP32, tag="ps")
                            for sc in (0, 1):
                                o = psD[:, sc, 0:h]
                                nc.tensor.matmul(o, ident[:], cv(sc), start=True, stop=False)
                                nc.tensor.matmul(o, nident[:], av(sc), start=False, stop=True)
                            r0 = rpool.tile([P, 2, 512], FP16, tag="r0")
                            nc.scalar.activation(r0[:, :, 0:h], psD[:, :, 0:h], Relu)

                            psE1 = pspool.tile([P, 2, 512], FP32, tag="ps")
                            for sc in (0, 1):
                                o = psE1[:, sc, 0:h]
                                nc.tensor.matmul(o, ident[:], cv(sc), start=True, stop=False)
                                nc.tensor.matmul(o, nident[:], bv(sc), start=False, stop=False)
                                nc.tensor.matmul(o, nident[:], r0[:, sc, 0:h], start=False, stop=True)
                            r1 = rpool.tile([P, 2, 512], FP16, tag="r1")
                            nc.scalar.activation(r1[:, :, 0:h], psE1[:, :, 0:h], Relu)

                            psE2 = pspool.tile([P, 2, 512], FP32, tag="ps")
                            for sc in (0, 1):
                                o = psE2[:, sc, 0:h]
                                nc.tensor.matmul(o, ident[:], bv(sc), start=True, stop=False)
                                nc.tensor.matmul(o, nident[:], av(sc), start=False, stop=False)
                                nc.tensor.matmul(o, nident[:], r0[:, sc, 0:h], start=False, stop=True)
                            r2 = rpool.tile([P, 2, 512], FP16, tag="r2")
                            nc.scalar.activation(r2[:, :, 0:h], psE2[:, :, 0:h], Relu)

                            psO = pspool.tile([P, 2, 512], FP32, tag="ps")
                            for sc in (0, 1):
                                o = psO[:, sc, 0:h]
                                nc.tensor.matmul(o, ident[:], bv(sc), start=True, stop=False)
                                nc.tensor.matmul(o, ident[:], r1[:, sc, 0:h], start=False, stop=False)
                                nc.tensor.matmul(o, nident[:], r2[:, sc, 0:h], start=False, stop=True)
                            # out centers -> otp[:, j-rd, 2 : 2+m] (one-elem store shift)
                            od = otp[:, j - rd, 2 : 2 + m].rearrange("p (s n) -> p s n", s=2)
                            nc.scalar.activation(od, psO[:, :, 0:h], Copy)
                        # edges for PE rows (ACT keeps sole ownership of otp)
                        nc.scalar.copy(
                            out=otp[:, :, 1 : Wo : W - 1], in_=x3[:, rd:r, 0 : W : W - 1]
                        )
                        if do_dma is True:
                            nc.sync.dma_start(out=dst[:, rd:r], in_=otp[:, :, 1 : W + 1])
    nc.compile()
    return nc


# default builder: the hybrid DVE + PE/ACT variant
build_program = build_program3


def kernel(x: np.ndarray) -> np.ndarray:
    return run_sharded(x, repeats=1)



# revision 19
# speedup vs baseline: 1.0480x; 1.0480x over previous
"""Trainium2 Bass kernel for nn_MedianPool2d (K=3, stride=1, same-pad along W).

The reference op is a width-wise median-of-3 with replicate padding:
    out[..., w] = median(x[..., w-1], x[..., w], x[..., w+1])   (clamped at edges)

Strategy (fp16 internal; gate is rel_err < 2e-2, we land at ~4.7e-4):
  - Shard batch across 8 NeuronCores (4 batches/core), fully data parallel.
    Host converts fp32 <-> fp16 so the device moves half the HBM bytes.
  - The DVE's 4-op min/max network (2 fp16/cycle/lane in 2x_1P mode) is a hard
    ~136us/core floor for all 8.4M elements: tensor_tensor has no 4x uop, no
    dual-output uop, custom DVE Specs run at 1x, GPSIMD is locked out of its
    shared SBUF port during every 2-port DVE op, and ScalarE has no two-tensor
    ops. So the kernel splits rows between two independent median pipelines:
      * DVE path (52/64 of rows): lo=min(x,S), hi=max(x,S), tm=min(hi,x+2),
        out=max(lo,tm), with S=x+1 an ACT shift copy that keeps every TT
        operand 4B-aligned (2x mode needs 16-bit + step 1 + 4B-aligned APs).
        Ops use per-row 3D APs [P, r, W-2] (row stride 2048B keeps alignment),
        so no cross-row seam garbage and no seam fixups.
      * PE/ACT path (12/64 of rows), on the otherwise-idle TensorE + spare
        ScalarE:  r0 = relu(c - a);  out = b + relu(c-b-r0) - relu(b-a-r0)
        (= clamp(b, min(a,c), max(a,c)) = median3). Linear terms are identity
        matmuls (+I/-I [128x128] fp16 consts) accumulated in PSUM, 11 matmuls
        per row in 511-wide chunks; relus + final fp32->fp16 cast-copy are 4
        ACT instrs per row reading PSUM. 22% of rows costs PE ~5.9us/row-unit
        and ACT ~4us, balancing DVE ~2.1us and ACT-copy ~0.9us per unit.
  - Engine budgets/core (sustained): DVE ~109, ACT ~100, PE ~64, DMA ~108
    (load+store 32MB at ~295GB/s sustained HBM; ~350GB/s burst).
  - The +1 output shift is absorbed by the store DMAs (byte-granular): ot rows
    are W+2 long so the DVE/ACT write dst starts 4B-aligned and the store
    reads at a one-element offset.  ot_dve and ot_pe are separate tiles with
    a single writer engine each (HWDGE store DMAs allow only one sem wait).
  - tc.high_priority() on the ACT shift copies and load DMAs: without it the
    shift copy queues behind the aux path's relu bursts on ACT and stalls the
    DVE (~25us). Deep xt buffering (bufs=6) rides out DMA/thermal jitter.
  - Timing is measured with the repeats-slope method; per-rep time grows with
    program length (throttling: ~88us/rep over reps 1-33, ~115 over 1-257,
    ~125 sustained at 129-257), so tile-schedule ramps amortize away and only
    engine-busy work matters.
"""

import numpy as np

import concourse.bacc as bacc
import concourse.bass as bass
import concourse.mybir as mybir
import concourse.tile as tile
from concourse.alu_op_type import AluOpType
from concourse.bass_utils import run_bass_kernel_spmd

N_CORES = 8
B, C, H, W = 32, 1, 2048, 1024
P = 128
ROWS = (B // N_CORES) * C * H  # 8192 rows per core
FP16 = mybir.dt.float16


DEFAULT_RS = (2, 2, 4, 8, 8, 8, 8, 8, 8, 4, 2, 2)  # sums to 64 row-units


def build_program(
    repeats: int = 1,
    rs: tuple = DEFAULT_RS,       # tile schedule in row-units (each unit = 128x1024)
    do_compute: bool = True,
    do_dma=True,                  # True / False / "load" (skip stores)
    copy_engine: str = "scalar",   # engine for the S shift copy
    dual_ring: bool = False,       # alternate store DMAs onto the ACT ring
    dual_load: bool = False,       # alternate load DMAs onto the ACT ring
    fixups: bool = True,           # seam/edge fixups (disable for timing only)
    bufs: tuple = (4, 2, 1, 1, 1, 3),
) -> bass.Bass:
    assert sum(rs) == ROWS // P
    nc = bacc.Bacc("TRN2", target_bir_lowering=False, debug=False)
    x_d = nc.dram_tensor("x", [ROWS, W], FP16, kind="ExternalInput").ap()
    y_d = nc.dram_tensor("y", [ROWS, W], FP16, kind="ExternalOutput").ap()

    with tile.TileContext(nc) as tc:
        with (
            tc.tile_pool(name="xt", bufs=bufs[0]) as xpool,
            tc.tile_pool(name="st", bufs=bufs[1]) as spool,
            tc.tile_pool(name="lo", bufs=bufs[2]) as lpool,
            tc.tile_pool(name="hi", bufs=bufs[3]) as hpool,
            tc.tile_pool(name="tm", bufs=bufs[4]) as tpool,
            tc.tile_pool(name="ot", bufs=bufs[5]) as opool,
        ):
            for _rep in range(repeats):
                row0 = 0
                for t, r in enumerate(rs):
                    L = r * W
                    n = L - 2  # even op length
                    rows = slice(row0 * P, (row0 + r) * P)
                    row0 += r
                    src = x_d[rows, :].rearrange("(p r) w -> p (r w)", p=P)
                    dst = y_d[rows, :].rearrange("(p r) w -> p (r w)", p=P)

                    xt = xpool.tile([P, L], FP16, tag="xt")
                    if do_dma:
                        leng = nc.scalar if (dual_load and t % 2) else nc.sync
                        leng.dma_start(out=xt[:], in_=src)

                    if not do_compute:
                        if do_dma is True:
                            eng = nc.scalar if (dual_ring and t % 2) else nc.sync
                            eng.dma_start(out=dst, in_=xt[:])
                        continue

                    st = spool.tile([P, L], FP16, tag="st")
                    # S[i] = x[i+1]; ops only read S[0 : L-2]
                    ceng = {"scalar": nc.scalar, "gpsimd": nc.gpsimd,
                            "vector": nc.vector}[copy_engine]
                    if copy_engine == "scalar":
                        ceng.copy(out=st[:, 0 : L - 1], in_=xt[:, 1:L])
                    else:
                        ceng.tensor_copy(out=st[:, 0 : L - 1], in_=xt[:, 1:L])

                    lo = lpool.tile([P, n], FP16, tag="lo")
                    hi = hpool.tile([P, n], FP16, tag="hi")
                    tm = tpool.tile([P, n], FP16, tag="tm")
                    ot = opool.tile([P, L + 2], FP16, tag="ot")

                    eng = nc.vector
                    eng.tensor_tensor(out=lo[:], in0=xt[:, 0:n], in1=st[:, 0:n], op=AluOpType.min)
                    eng.tensor_tensor(out=hi[:], in0=xt[:, 0:n], in1=st[:, 0:n], op=AluOpType.max)
                    eng.tensor_tensor(out=tm[:], in0=hi[:], in1=xt[:, 2 : 2 + n], op=AluOpType.min)
                    # ot[2+i] = c[i] = O[i+1], i = 0..n-1  (aligned dst)
                    eng.tensor_tensor(out=ot[:, 2 : 2 + n], in0=lo[:], in1=tm[:], op=AluOpType.max)

                    # row-seam fixups: for k=1..r-1, O[kW-1]=x[kW-1], O[kW]=x[kW]
                    #   -> ot[kW : kW+2] = xt[kW-1 : kW+1]
                    if fixups and r > 1:
                        seam_dst = ot[:, W : (r - 1) * W + W : 1].rearrange(
                            "p (k w) -> p k w", w=W
                        )[:, :, 0:2]
                        seam_src = xt[:, W - 1 : (r - 1) * W + W - 1 : 1].rearrange(
                            "p (k w) -> p k w", w=W
                        )[:, :, 0:2]
                        eng.tensor_copy(out=seam_dst, in_=seam_src)
                    # edges: O[0]=x[0] -> ot[1]; O[L-1]=x[L-1] -> ot[L]
                    if fixups:
                        eng.tensor_copy(
                            out=ot[:, 1 : L + 1 : L - 1], in_=xt[:, 0 : L : L - 1]
                        )

                    if do_dma is True:
                        deng = nc.scalar if (dual_ring and t % 2) else nc.sync
                        deng.dma_start(out=dst, in_=ot[:, 1 : L + 1])
    nc.compile()
    return nc


def build_program2(
    repeats: int = 1,
    rs: tuple = (8,) * 8,
    do_dma=True,                  # True / False / "load"
    fixup_engine: str = "vector",  # engine for the per-row edge fixup
    store_ring: str = "sync",
    load_ring: str = "sync",
    bufs: tuple = (4, 2, 1, 1, 1, 3),
) -> bass.Bass:
    """Per-row 3D-AP variant: no seam fixups, one merged edge fixup per tile.

    Views are [P, r, *] with row stride W (2048B, 4B-aligned), so every TT
    operand keeps 2x packed mode. Output tile rows are padded to W+2 so the
    DVE write dst ([:, :, 2:W]) starts 4B-aligned; the store DMA reads at a
    one-element offset to undo the shift.
    """
    Wo = W + 2  # padded ot row length (stride 2052B, 4B-multiple)
    assert sum(rs) == ROWS // P
    nc = bacc.Bacc("TRN2", target_bir_lowering=False, debug=False)
    x_d = nc.dram_tensor("x", [ROWS, W], FP16, kind="ExternalInput").ap()
    y_d = nc.dram_tensor("y", [ROWS, W], FP16, kind="ExternalOutput").ap()
    rings = {"sync": nc.sync, "scalar": nc.scalar}

    with tile.TileContext(nc) as tc:
        with (
            tc.tile_pool(name="xt", bufs=bufs[0]) as xpool,
            tc.tile_pool(name="st", bufs=bufs[1]) as spool,
            tc.tile_pool(name="lo", bufs=bufs[2]) as lpool,
            tc.tile_pool(name="hi", bufs=bufs[3]) as hpool,
            tc.tile_pool(name="tm", bufs=bufs[4]) as tpool,
            tc.tile_pool(name="ot", bufs=bufs[5]) as opool,
        ):
            for _rep in range(repeats):
                row0 = 0
                for t, r in enumerate(rs):
                    L = r * W
                    m = W - 2  # interior outputs per row
                    rows = slice(row0 * P, (row0 + r) * P)
                    row0 += r
                    src = x_d[rows, :].rearrange("(p r) w -> p (r w)", p=P)
                    dst = y_d[rows, :].rearrange("(p r) w -> p r w", p=P)

                    xt = xpool.tile([P, L], FP16, tag="xt")
                    if do_dma:
                        rings[load_ring].dma_start(out=xt[:], in_=src)
                    x3 = xt[:].rearrange("p (r w) -> p r w", w=W)

                    st = spool.tile([P, L], FP16, tag="st")
                    # flat shift: S[i] = x[i+1] (cross-row values unused)
                    nc.scalar.copy(out=st[:, 0 : L - 1], in_=xt[:, 1:L])
                    s3 = st[:].rearrange("p (r w) -> p r w", w=W)

                    lo = lpool.tile([P, r, m], FP16, tag="lo")
                    hi = hpool.tile([P, r, m], FP16, tag="hi")
                    tm = tpool.tile([P, r, m], FP16, tag="tm")
                    ot = opool.tile([P, r, Wo], FP16, tag="ot")

                    eng = nc.vector
                    # centers w=1..W-2: a=x[w-1], b=x[w], c=x[w+1]
                    eng.tensor_tensor(out=lo[:], in0=x3[:, :, 0:m], in1=s3[:, :, 0:m], op=AluOpType.min)
                    eng.tensor_tensor(out=hi[:], in0=x3[:, :, 0:m], in1=s3[:, :, 0:m], op=AluOpType.max)
                    eng.tensor_tensor(out=tm[:], in0=hi[:], in1=x3[:, :, 2:W], op=AluOpType.min)
                    # ot[:, :, 2+i] = O[i+1], i=0..m-1 (aligned dst)
                    eng.tensor_tensor(out=ot[:, :, 2 : 2 + m], in0=lo[:], in1=tm[:], op=AluOpType.max)

                    # edges per row: O[0]=x[0] -> ot[:, :, 1]; O[W-1]=x[W-1] -> ot[:, :, W]
                    feng = {"vector": nc.vector, "gpsimd": nc.gpsimd}.get(fixup_engine)
                    edst = ot[:, :, 1 : Wo : W - 1]      # [P, r, 2]
                    esrc = x3[:, :, 0 : W : W - 1]       # [P, r, 2]
                    if feng is not None:
                        feng.tensor_copy(out=edst, in_=esrc)
                    else:
                        nc.scalar.copy(out=edst, in_=esrc)

                    if do_dma is True:
                        rings[store_ring].dma_start(out=dst, in_=ot[:, :, 1 : W + 1])
    nc.compile()
    return nc


_NC_CACHE: dict = {}


def prep_shards(x: np.ndarray) -> np.ndarray:
    """[B,C,H,W] fp32 -> [N_CORES, ROWS, W] fp16 (contiguous)."""
    x = np.asarray(x)
    assert x.shape == (B, C, H, W), x.shape
    return np.ascontiguousarray(x.reshape(N_CORES, ROWS, W)).astype(np.float16)


def run_sharded(x: np.ndarray, repeats: int = 1, builder=None, **knobs) -> np.ndarray:
    build = builder or build_program
    x16 = prep_shards(x)
    key = (build.__name__, repeats, tuple(sorted(knobs.items())))
    nc = _NC_CACHE.get(key)
    if nc is None:
        nc = _NC_CACHE[key] = build(repeats=repeats, **knobs)
    in_maps = [{"x": x16[i]} for i in range(N_CORES)]
    res = run_bass_kernel_spmd(nc, in_maps, core_ids=list(range(N_CORES))).results
    out = np.stack([res[i]["y"] for i in range(N_CORES)], axis=0)
    return out.reshape(B, C, H, W).astype(np.float32)


def build_program3(
    repeats: int = 1,
    rs: tuple = (8,) * 8,
    rps: tuple = (2, 1, 2, 1, 2, 1, 2, 1),  # PE-path rows per tile
    do_dma=True,
    bufs: tuple = (6, 2, 1, 1, 1, 2, 3),
    psum_bufs: int = 4,
    aux_bufs: int = 2,
    hp_loads: bool = True,  # prioritize load DMAs for the scheduler
    inplace_tm: bool = True,  # write tm over hi (frees the tm pool)
) -> bass.Bass:
    """Hybrid: DVE min/max network on rd rows + PE/ACT relu-formula on rp rows.

    PE path per row (interior centers w=1..W-2, split in two 511-chunks):
        r0 = relu(c - a); out = b + relu(c - b - r0) - relu(b - a - r0)
    which equals clamp(b, min(a,c), max(a,c)) = median3. All linear terms are
    identity matmuls accumulated in PSUM; relus/final-cast run on ScalarE.
    ot_pe has a single writer engine (ACT); ot_d's is the DVE.
    """
    import concourse.masks as masks

    Wo = W + 2
    m = W - 2
    h = m // 2  # 511
    assert sum(rs) == ROWS // P and len(rps) == len(rs)
    nc = bacc.Bacc("TRN2", target_bir_lowering=False, debug=False)
    x_d = nc.dram_tensor("x", [ROWS, W], FP16, kind="ExternalInput").ap()
    y_d = nc.dram_tensor("y", [ROWS, W], FP16, kind="ExternalOutput").ap()
    FP32 = mybir.dt.float32
    Relu = mybir.ActivationFunctionType.Relu
    Copy = mybir.ActivationFunctionType.Copy

    with tile.TileContext(nc) as tc:
        with (
            tc.tile_pool(name="const", bufs=1) as cpool,
            tc.tile_pool(name="xt", bufs=bufs[0]) as xpool,
            tc.tile_pool(name="st", bufs=bufs[1]) as spool,
            tc.tile_pool(name="lo", bufs=bufs[2]) as lpool,
            tc.tile_pool(name="hi", bufs=bufs[3]) as hpool,
            tc.tile_pool(name="tm", bufs=bufs[4]) as tpool,
            tc.tile_pool(name="ot", bufs=bufs[5]) as opool,
            tc.tile_pool(name="op", bufs=bufs[6]) as oppool,
            tc.tile_pool(name="ps", bufs=psum_bufs, space="PSUM") as pspool,
            tc.tile_pool(name="rl", bufs=aux_bufs) as rpool,
        ):
            ident = cpool.tile([P, P], FP16, tag="ident")
            nident = cpool.tile([P, P], FP16, tag="nident")
            masks.make_identity(nc, ident[:])
            nc.vector.tensor_scalar_mul(nident[:], ident[:], -1.0)

            for _rep in range(repeats):
                row0 = 0
                for t, (r, rp) in enumerate(zip(rs, rps)):
                    rd = r - rp
                    L = r * W
                    rows = slice(row0 * P, (row0 + r) * P)
                    row0 += r
                    src = x_d[rows, :].rearrange("(p r) w -> p (r w)", p=P)
                    dst = y_d[rows, :].rearrange("(p r) w -> p r w", p=P)

                    xt = xpool.tile([P, L], FP16, tag="xt")
                    if do_dma:
                        if hp_loads:
                            with tc.high_priority():
                                nc.sync.dma_start(out=xt[:], in_=src)
                        else:
                            nc.sync.dma_start(out=xt[:], in_=src)
                    x3 = xt[:].rearrange("p (r w) -> p r w", w=W)

                    # ---- DVE path: rows [0, rd) ----
                    if rd > 0:
                        Ld = rd * W
                        st = spool.tile([P, Ld], FP16, tag="st")
                        # priority 0: never let the DVE-feeding shift copy queue
                        # behind the aux path's relu bursts on the ACT engine
                        with tc.high_priority():
                            nc.scalar.copy(out=st[:, 0 : Ld], in_=xt[:, 1 : Ld + 1])
                        s3 = st[:].rearrange("p (r w) -> p r w", w=W)

                        lo = lpool.tile([P, rd, m], FP16, tag="lo")
                        hi = hpool.tile([P, rd, m], FP16, tag="hi")
                        tm = hi if inplace_tm else tpool.tile([P, rd, m], FP16, tag="tm")
                        ot = opool.tile([P, rd, Wo], FP16, tag="ot")

                        eng = nc.vector
                        xd = x3[:, 0:rd]
                        sd = s3[:, 0:rd]
                        eng.tensor_tensor(out=lo[:], in0=xd[:, :, 0:m], in1=sd[:, :, 0:m], op=AluOpType.min)
                        eng.tensor_tensor(out=hi[:], in0=xd[:, :, 0:m], in1=sd[:, :, 0:m], op=AluOpType.max)
                        eng.tensor_tensor(out=tm[:], in0=hi[:], in1=xd[:, :, 2:W], op=AluOpType.min)
                        eng.tensor_tensor(out=ot[:, :, 2 : 2 + m], in0=lo[:], in1=tm[:], op=AluOpType.max)
                        eng.tensor_copy(out=ot[:, :, 1 : Wo : W - 1], in_=xd[:, :, 0 : W : W - 1])
                        if do_dma is True:
                            nc.sync.dma_start(out=dst[:, 0:rd], in_=ot[:, :, 1 : W + 1])

                    # ---- PE path: rows [rd, r) ----
                    if rp > 0:
                        otp = oppool.tile([P, rp, Wo], FP16, tag="otp")
                        for j in range(rd, r):
                            xr = x3[:, j]          # [P, W]
                            # sub-chunk views: centers w in [1+h*sc, 1+h*(sc+1))
                            def av(sc):
                                return xr[:, h * sc : h * sc + h]
                            def bv(sc):
                                return xr[:, 1 + h * sc : 1 + h * sc + h]
                            def cv(sc):
                                return xr[:, 2 + h * sc : 2 + h * sc + h]

                            psD = pspool.tile([P, 2, 512], FP32, tag="ps")
                            for sc in (0, 1):
                                o = psD[:, sc, 0:h]
                                nc.tensor.matmul(o, ident[:], cv(sc), start=True, stop=False)
                                nc.tensor.matmul(o, nident[:], av(sc), start=False, stop=True)
                            r0 = rpool.tile([P, 2, 512], FP16, tag="r0")
                            nc.scalar.activation(r0[:, :, 0:h], psD[:, :, 0:h], Relu)

                            psE1 = pspool.tile([P, 2, 512], FP32, tag="ps")
                            for sc in (0, 1):
                                o = psE1[:, sc, 0:h]
                                nc.tensor.matmul(o, ident[:], cv(sc), start=True, stop=False)
                                nc.tensor.matmul(o, nident[:], bv(sc), start=False, stop=False)
                                nc.tensor.matmul(o, nident[:], r0[:, sc, 0:h], start=False, stop=True)
                            r1 = rpool.tile([P, 2, 512], FP16, tag="r1")
                            nc.scalar.activation(r1[:, :, 0:h], psE1[:, :, 0:h], Relu)

                            psE2 = pspool.tile([P, 2, 512], FP32, tag="ps")
                            for sc in (0, 1):
                                o = psE2[:, sc, 0:h]
                                nc.tensor.matmul(o, ident[:], bv(sc), start=True, stop=False)
                                nc.tensor.matmul(o, nident[:], av(sc), start=False, stop=False)
                                nc.tensor.matmul(o, nident[:], r0[:, sc, 0:h], start=False, stop=True)
                            r2 = rpool.tile([P, 2, 512], FP16, tag="r2")
                            nc.scalar.activation(r2[:, :, 0:h], psE2[:, :, 0:h], Relu)

                            psO = pspool.tile([P, 2, 512], FP32, tag="ps")
                            for sc in (0, 1):
                                o = psO[:, sc, 0:h]
                                nc.tensor.matmul(o, ident[:], bv(sc), start=True, stop=False)
                                nc.tensor.matmul(o, ident[:], r1[:, sc, 0:h], start=False, stop=False)
                                nc.tensor.matmul(o, nident[:], r2[:, sc, 0:h], start=False, stop=True)
                            # out centers -> otp[:, j-rd, 2 : 2+m] (one-elem store shift)
                            od = otp[:, j - rd, 2 : 2 + m].rearrange("p (s n) -> p s n", s=2)
                            nc.scalar.activation(od, psO[:, :, 0:h], Copy)
                        # edges for PE rows (ACT keeps sole ownership of otp)
                        nc.scalar.copy(
                            out=otp[:, :, 1 : Wo : W - 1], in_=x3[:, rd:r, 0 : W : W - 1]
                        )
                        if do_dma is True:
                            nc.sync.dma_start(out=dst[:, rd:r], in_=otp[:, :, 1 : W + 1])
    nc.compile()
    return nc


# default builder: the hybrid DVE + PE/ACT variant
build_program = build_program3


def kernel(x: np.ndarray) -> np.ndarray:
    return run_sharded(x, repeats=1)

